# revision 4
# baseline (speedup 1.0000x reference)
"""LFISTA Trainium2 kernel: transposed-state layout, zero PE transposes.

Data-parallel over batch (8 cores x 128 rows). All state lives in SBUF in
[S, B] layout (partition = s mod 128, free = (s_chunk, b)), so both matmuls
use the weights as the stationary operand and the activations as the moving
operand directly -- no per-iteration activation transposes, no weight
streaming (both the W^T and W/L layouts are SBUF-resident in bf16; FWL makes
the per-matmul LDWEIGHTS free, giving the 56 ns/MM stream floor at N=128).

Per iteration: m1T = W @ ythT (stationary = W^T blocks), elementwise chain,
m2T = (W/L)^T @ zT (stationary = W rows). Matmuls are emitted half-by-half
(8 s-blocks per half, k inner) so DVE work on one half overlaps PE work on
the other; the theta-side threshold/momentum runs at psum-bank granularity
so yth chunks are ready ahead of the next m1's consumption. All elementwise
on DVE (GPSIMD tensor ops are ~20x slower and contend for the DVE SBUF
port). Accuracy-critical tensors (src, q/vth, clip, res) are f32; the rest
of the state is bf16 (fp16 overflows: this LFISTA instance diverges by
design, values reach ~1e13 by iteration 15).
"""
import math
import numpy as np

B = 1024
S = 2048
ITERS = 16
NCORES = 8
BC = B // NCORES   # 128
NCH = S // 128     # 16 chunks
HCH = NCH // 2     # 8 chunks per half


def _momentum_coeffs(n):
    cks = []
    t = 1.0
    for _ in range(n):
        t_new = (1.0 + math.sqrt(1.0 + 4.0 * t * t)) / 2.0
        cks.append((t - 1.0) / t_new)
        t = t_new
    return cks


def _build(invL, thresh, cks):
    import concourse.bacc as bacc
    import concourse.mybir as mybir
    from concourse.tile import TileContext

    dt = mybir.dt
    ALU = mybir.AluOpType
    f32, f16 = dt.float32, dt.bfloat16

    nc = bacc.Bacc("TRN2", target_bir_lowering=False, debug=False)

    # host-packed layouts: [128, NCH, X] with partition = s mod 128
    src_d = nc.dram_tensor("src", [128, NCH * BC], f32, kind="ExternalInput")
    yin_d = nc.dram_tensor("yin", [128, NCH * BC], f16, kind="ExternalInput")
    wr_d = nc.dram_tensor("wr", [128, NCH * S], f16, kind="ExternalInput")  # W^T rows
    wc_d = nc.dram_tensor("wc", [128, NCH * S], f16, kind="ExternalInput")  # (W/L) rows
    out_d = nc.dram_tensor("out", [128, 2 * NCH * BC], f16, kind="ExternalOutput")

    with TileContext(nc) as tc:
        with tc.tile_pool(name="wpool", bufs=1) as wp, \
             tc.tile_pool(name="state", bufs=1) as st, \
             tc.tile_pool(name="pmm", bufs=1, space="PSUM") as pp:

            wr_sb = wp.tile([128, NCH, S], f16, name="wr_sb")
            wc_sb = wp.tile([128, NCH, S], f16, name="wc_sb")

            srcT = st.tile([128, NCH, BC], f32, name="srcT")
            yinT = st.tile([128, NCH, BC], f16, name="yinT")
            ynd = st.tile([128, NCH, BC], f16, name="ynd")    # yin - ydl
            ydl = st.tile([128, NCH, BC], f16, name="ydl")
            yth = st.tile([128, NCH, BC], f16, name="yth")
            res = st.tile([128, NCH, BC], f32, name="res")
            z16 = st.tile([128, NCH, BC], f16, name="z16")
            qv = st.tile([128, NCH, BC], f32, name="qv")      # q, then vth
            cth = st.tile([128, NCH, BC], f32, name="cth")
            vdl = st.tile([128, NCH, BC], f16, name="vdl")
            cdl = st.tile([128, NCH, BC], f16, name="cdl")
            xthA = st.tile([128, NCH, BC], f16, name="xthA")
            xthB = st.tile([128, NCH, BC], f16, name="xthB")
            xdlA = st.tile([128, NCH, BC], f16, name="xdlA")
            xdlB = st.tile([128, NCH, BC], f16, name="xdlB")

            def halfsl(t, h):
                return t[:, h * HCH:(h + 1) * HCH, :]

            # small inputs on the ACT HWDGE ring, weights on the SP ring:
            # the two rings drain in parallel, so srcT/yinT (needed by the
            # iter-0 elementwise) don't sit behind 16 MB of weights; per-half
            # pieces so iter-0's z16[h0] unblocks as early as possible
            hb = HCH * BC
            for h in range(2):
                nc.scalar.dma_start(halfsl(srcT, h), src_d[:, h * hb:(h + 1) * hb])
                nc.scalar.dma_start(halfsl(yinT, h), yin_d[:, h * hb:(h + 1) * hb])
            for c in range(NCH):
                nc.sync.dma_start(wc_sb[:, c, :], wc_d[:, c * S:(c + 1) * S])
            for c in range(NCH):
                nc.sync.dma_start(wr_sb[:, c, :], wr_d[:, c * S:(c + 1) * S])

            def mm_half(pm, w_sb, rhs_t, h):
                # NOTE: start=True clears has_written for the WHOLE bank, so
                # only the first matmul touching each bank may set it.
                for kb in range(NCH):
                    for sb in range(h * HCH, (h + 1) * HCH):
                        nc.tensor.matmul(
                            pm[sb // 4][:, (sb % 4) * BC:(sb % 4 + 1) * BC],
                            lhsT=w_sb[:, kb, sb * 128:(sb + 1) * 128],
                            rhs=rhs_t[:, kb, :],
                            start=(kb == 0 and sb % 4 == 0),
                            stop=(kb == NCH - 1))

            def theta_eval(pm2, x_new, first, x_old=None, ck=None):
                # vth = yth + m2 (or just m2 at iter 0); soft-threshold;
                # quarter (psum-bank) granularity so yth chunks are ready
                # progressively, ahead of the next m1's chunk consumption
                for g in range(4):
                    sl = lambda t: t[:, 4 * g:4 * g + 4, :]
                    if first:
                        nc.vector.tensor_copy(out=sl(qv), in_=pm2[g][:])
                    else:
                        nc.vector.tensor_tensor(
                            out=sl(qv), in0=sl(yth), in1=pm2[g][:], op=ALU.add)
                    nc.vector.tensor_scalar(
                        out=sl(cth), in0=sl(qv),
                        scalar1=-thresh, scalar2=thresh,
                        op0=ALU.max, op1=ALU.min)
                    nc.vector.tensor_tensor(
                        out=sl(x_new), in0=sl(qv), in1=sl(cth),
                        op=ALU.subtract)
                    if ck is not None:
                        nc.vector.tensor_tensor(
                            out=sl(yth), in0=sl(x_new), in1=sl(x_old),
                            op=ALU.subtract)
                        nc.vector.scalar_tensor_tensor(
                            out=sl(yth), in0=sl(yth), scalar=ck,
                            in1=sl(x_new), op0=ALU.mult, op1=ALU.add)

            # ---------------- iteration 0 (y == 0) ----------------
            # res = yin ; z = src*yin ; vth = m2 ; vdl = yin/L
            for h in range(2):
                nc.vector.tensor_tensor(
                    out=halfsl(z16, h), in0=halfsl(srcT, h),
                    in1=halfsl(yinT, h), op=ALU.mult)
            pm2 = [pp.tile([128, 4 * BC], f32, name=f"p2{i}", tag=f"pm2_{i}")
                   for i in range(4)]
            for h in range(2):
                mm_half(pm2, wc_sb, z16, h)

            # delta half (DVE; GPSIMD elementwise is ~20x too slow and its
            # SBUF-port contention also stalls DVE)
            for h in range(2):
                nc.vector.tensor_scalar(
                    out=halfsl(vdl, h), in0=halfsl(yinT, h),
                    scalar1=invL, scalar2=None, op0=ALU.mult)
                nc.vector.tensor_scalar(
                    out=halfsl(cdl, h), in0=halfsl(vdl, h),
                    scalar1=-thresh, scalar2=thresh, op0=ALU.max, op1=ALU.min)
                nc.vector.tensor_tensor(
                    out=halfsl(xdlB, h), in0=halfsl(vdl, h),
                    in1=halfsl(cdl, h), op=ALU.subtract)
                # y1 = x1 (c0 = 0)
                nc.vector.tensor_copy(out=halfsl(ydl, h), in_=halfsl(xdlB, h))
                nc.vector.tensor_tensor(
                    out=halfsl(ynd, h), in0=halfsl(yinT, h),
                    in1=halfsl(ydl, h), op=ALU.subtract)

            theta_eval(pm2, xthB, first=True)
            for h in range(2):
                nc.vector.tensor_copy(out=halfsl(yth, h), in_=halfsl(xthB, h))

            x_old = [xthB, xdlB]
            x_new = [xthA, xdlA]

            # ---------------- iterations 1..15 ----------------
            for k in range(1, ITERS):
                ck = cks[k]
                last = (k == ITERS - 1)

                pm1 = [pp.tile([128, 4 * BC], f32, name=f"p1{i}", tag=f"pm1_{i}")
                       for i in range(4)]
                for h in range(2):
                    mm_half(pm1, wr_sb, yth, h)
                    # q = src*m1 ; res = ynd - q ; z = src*res
                    for i in (2 * h, 2 * h + 1):
                        nc.vector.tensor_tensor(
                            out=qv[:, 4 * i:4 * i + 4, :],
                            in0=srcT[:, 4 * i:4 * i + 4, :],
                            in1=pm1[i][:], op=ALU.mult)
                    nc.vector.tensor_tensor(
                        out=halfsl(res, h), in0=halfsl(ynd, h),
                        in1=halfsl(qv, h), op=ALU.subtract)
                    nc.vector.tensor_tensor(
                        out=halfsl(z16, h), in0=halfsl(srcT, h),
                        in1=halfsl(res, h), op=ALU.mult)

                pm2 = [pp.tile([128, 4 * BC], f32, name=f"p2{i}", tag=f"pm2_{i}")
                       for i in range(4)]
                for h in range(2):
                    mm_half(pm2, wc_sb, z16, h)

                # delta half (DVE, overlaps m2 on PE)
                for h in range(2):
                    nc.vector.scalar_tensor_tensor(
                        out=halfsl(vdl, h), in0=halfsl(res, h), scalar=invL,
                        in1=halfsl(ydl, h), op0=ALU.mult, op1=ALU.add)
                    nc.vector.tensor_scalar(
                        out=halfsl(cdl, h), in0=halfsl(vdl, h),
                        scalar1=-thresh, scalar2=thresh,
                        op0=ALU.max, op1=ALU.min)
                    nc.vector.tensor_tensor(
                        out=halfsl(x_new[1], h), in0=halfsl(vdl, h),
                        in1=halfsl(cdl, h), op=ALU.subtract)
                if not last:
                    for h in range(2):
                        nc.vector.tensor_tensor(
                            out=halfsl(ydl, h), in0=halfsl(x_new[1], h),
                            in1=halfsl(x_old[1], h), op=ALU.subtract)
                        nc.vector.scalar_tensor_tensor(
                            out=halfsl(ydl, h), in0=halfsl(ydl, h), scalar=ck,
                            in1=halfsl(x_new[1], h), op0=ALU.mult, op1=ALU.add)
                        nc.vector.tensor_tensor(
                            out=halfsl(ynd, h), in0=halfsl(yinT, h),
                            in1=halfsl(ydl, h), op=ALU.subtract)
                else:
                    # final xdl DMA overlaps the remaining theta work
                    nc.sync.dma_start(out_d[:, NCH * BC:], x_new[1][:])

                theta_eval(pm2, x_new[0], first=False,
                           x_old=x_old[0], ck=None if last else ck)

                x_old, x_new = x_new, x_old

            # final x is in x_old after the swap (xdl already written)
            nc.sync.dma_start(out_d[:, :NCH * BC], x_old[0][:])

    nc.finalize()
    return nc


_CACHE = {}


def _pack(a, nch, dtype=None):
    # [nch*128, X] -> [128, nch, X] with partition = row mod 128
    import ml_dtypes
    x = np.ascontiguousarray(a).reshape(nch, 128, a.shape[1])
    out = np.ascontiguousarray(
        x.transpose(1, 0, 2).reshape(128, nch * a.shape[1]))
    return out.astype(dtype if dtype is not None else ml_dtypes.bfloat16)


def kernel(src, Y, W, alpha):
    src = np.asarray(src)
    Y = np.asarray(Y)
    W = np.asarray(W)
    alpha = np.asarray(alpha)

    from concourse.bass_utils import run_bass_kernel_spmd

    G = W.astype(np.float64).T @ W.astype(np.float64)
    L = float(np.linalg.eigvalsh(G)[-1])
    invL = float(np.float32(1.0 / L))
    thresh = float(np.float32(float(alpha.reshape(-1)[0]) / L * 0.5))
    cks = _momentum_coeffs(ITERS)

    key = (invL, thresh)
    if key not in _CACHE:
        _CACHE[key] = _build(invL, thresh, cks)
    nc = _CACHE[key]

    wrP = _pack(np.ascontiguousarray(W.T).astype(np.float32), NCH)
    wcP = _pack((W / L).astype(np.float32), NCH)
    src2 = src.reshape(B, S).astype(np.float32)
    Y2 = Y.reshape(B, S).astype(np.float32)

    in_maps = []
    for c in range(NCORES):
        sl = slice(c * BC, (c + 1) * BC)
        in_maps.append({
            "src": _pack(np.ascontiguousarray(src2[sl].T), NCH, np.float32),
            "yin": _pack(np.ascontiguousarray(Y2[sl].T), NCH),
            "wr": wrP,
            "wc": wcP,
        })

    r = run_bass_kernel_spmd(nc, in_maps, core_ids=list(range(NCORES)))
    global LAST_RESULTS
    LAST_RESULTS = r

    outs = []
    for c in range(NCORES):
        o = np.asarray(r.results[c]["out"]).astype(np.float32)
        o = o.reshape(128, 2, NCH, BC).transpose(1, 2, 0, 3).reshape(2 * S, BC)
        outs.append(o.T)  # [BC, 2S]
    out = np.concatenate(outs, axis=0)
    return out.reshape(B, 2 * S, 1).astype(np.float32)


LAST_RESULTS = None


# revision 8
# speedup vs baseline: 1.0136x; 1.0136x over previous
"""LFISTA Trainium2 kernel: transposed-state layout, zero PE transposes.

Data-parallel over batch (8 cores x 128 rows). All state lives in SBUF in
[S, B] layout (partition = s mod 128, free = (s_chunk, b)), so both matmuls
use the weights as the stationary operand and the activations as the moving
operand directly -- no per-iteration activation transposes, no weight
streaming (both the W^T and W/L layouts are SBUF-resident in bf16; FWL makes
the per-matmul LDWEIGHTS free, giving the 56 ns/MM stream floor at N=128).

Per iteration: m1T = W @ ythT (stationary = W^T blocks), elementwise chain,
m2T = (W/L)^T @ zT (stationary = W rows). Matmuls are emitted half-by-half
(8 s-blocks per half, k inner) so DVE work on one half overlaps PE work on
the other; the theta-side threshold/momentum runs at psum-bank granularity
so yth chunks are ready ahead of the next m1's consumption. All elementwise
on DVE (GPSIMD tensor ops are ~20x slower and contend for the DVE SBUF
port). Accuracy-critical tensors (src, q/vth, clip, res) are f32; the rest
of the state is bf16 (fp16 overflows: this LFISTA instance diverges by
design, values reach ~1e13 by iteration 15).
"""
import math
import numpy as np

B = 1024
S = 2048
ITERS = 16
NCORES = 8
BC = B // NCORES   # 128
NCH = S // 128     # 16 chunks
HCH = NCH // 2     # 8 chunks per half


def _momentum_coeffs(n):
    cks = []
    t = 1.0
    for _ in range(n):
        t_new = (1.0 + math.sqrt(1.0 + 4.0 * t * t)) / 2.0
        cks.append((t - 1.0) / t_new)
        t = t_new
    return cks


def _build(invL, thresh, cks):
    import concourse.bacc as bacc
    import concourse.mybir as mybir
    from concourse.tile import TileContext

    dt = mybir.dt
    ALU = mybir.AluOpType
    f32, f16 = dt.float32, dt.bfloat16

    nc = bacc.Bacc("TRN2", target_bir_lowering=False, debug=False)

    # host-packed layouts: [128, NCH, X] with partition = s mod 128
    src_d = nc.dram_tensor("src", [128, NCH * BC], f32, kind="ExternalInput")
    yin_d = nc.dram_tensor("yin", [128, NCH * BC], f16, kind="ExternalInput")
    wr_d = nc.dram_tensor("wr", [128, NCH * S], f16, kind="ExternalInput")  # W^T rows
    wc_d = nc.dram_tensor("wc", [128, NCH * S], f16, kind="ExternalInput")  # (W/L) rows
    out_d = nc.dram_tensor("out", [128, 2 * NCH * BC], f16, kind="ExternalOutput")

    with TileContext(nc) as tc:
        with tc.tile_pool(name="wpool", bufs=1) as wp, \
             tc.tile_pool(name="state", bufs=1) as st, \
             tc.tile_pool(name="pmm", bufs=1, space="PSUM") as pp:

            wr_sb = wp.tile([128, NCH, S], f16, name="wr_sb")
            wc_sb = wp.tile([128, NCH, S], f16, name="wc_sb")

            srcT = st.tile([128, NCH, BC], f32, name="srcT")
            yinT = st.tile([128, NCH, BC], f16, name="yinT")
            ynd = st.tile([128, NCH, BC], f16, name="ynd")    # yin - ydl
            ydl = st.tile([128, NCH, BC], f16, name="ydl")
            yth = st.tile([128, NCH, BC], f16, name="yth")
            res = st.tile([128, NCH, BC], f32, name="res")
            z16 = st.tile([128, NCH, BC], f16, name="z16")
            qv = st.tile([128, NCH, BC], f32, name="qv")      # q, then vth
            cth = st.tile([128, NCH, BC], f32, name="cth")
            vdl = st.tile([128, NCH, BC], f16, name="vdl")
            cdl = st.tile([128, NCH, BC], f16, name="cdl")
            xthA = st.tile([128, NCH, BC], f16, name="xthA")
            xthB = st.tile([128, NCH, BC], f16, name="xthB")
            xdlA = st.tile([128, NCH, BC], f16, name="xdlA")
            xdlB = st.tile([128, NCH, BC], f16, name="xdlB")

            def halfsl(t, h):
                return t[:, h * HCH:(h + 1) * HCH, :]

            # small inputs on the ACT HWDGE ring, weights on the SP ring:
            # the two rings drain in parallel, so srcT/yinT (needed by the
            # iter-0 elementwise) don't sit behind 16 MB of weights; quarter
            # pieces so iter-0's z16 quarters unblock as early as possible
            qb = 4 * BC
            for g in range(4):
                nc.scalar.dma_start(srcT[:, 4 * g:4 * g + 4, :],
                                    src_d[:, g * qb:(g + 1) * qb])
                nc.scalar.dma_start(yinT[:, 4 * g:4 * g + 4, :],
                                    yin_d[:, g * qb:(g + 1) * qb])
            # weights split into column halves, h0 columns of every chunk
            # first: mm_half(h) only reads cols [h*1024, (h+1)*1024) of each
            # chunk, so this halves the DMA the first compute pass waits on
            HS = S // 2
            for w_sb, w_d in ((wc_sb, wc_d), (wr_sb, wr_d)):
                for hw in range(2):
                    for c in range(NCH):
                        nc.sync.dma_start(
                            w_sb[:, c, hw * HS:(hw + 1) * HS],
                            w_d[:, c * S + hw * HS:c * S + (hw + 1) * HS])

            def mm_half(pm, w_sb, rhs_t, h):
                # NOTE: start=True clears has_written for the WHOLE bank, so
                # only the first matmul touching each bank may set it.
                for kb in range(NCH):
                    for sb in range(h * HCH, (h + 1) * HCH):
                        nc.tensor.matmul(
                            pm[sb // 4][:, (sb % 4) * BC:(sb % 4 + 1) * BC],
                            lhsT=w_sb[:, kb, sb * 128:(sb + 1) * 128],
                            rhs=rhs_t[:, kb, :],
                            start=(kb == 0 and sb % 4 == 0),
                            stop=(kb == NCH - 1))

            def theta_eval(pm2, x_new, first, x_old=None, ck=None,
                           out_dma=False):
                # vth = yth + m2 (or just m2 at iter 0); soft-threshold;
                # quarter (psum-bank) granularity so yth chunks are ready
                # progressively, ahead of the next m1's chunk consumption
                for g in range(4):
                    sl = lambda t: t[:, 4 * g:4 * g + 4, :]
                    if first:
                        nc.vector.tensor_copy(out=sl(qv), in_=pm2[g][:])
                    else:
                        nc.vector.tensor_tensor(
                            out=sl(qv), in0=sl(yth), in1=pm2[g][:], op=ALU.add)
                    nc.vector.tensor_scalar(
                        out=sl(cth), in0=sl(qv),
                        scalar1=-thresh, scalar2=thresh,
                        op0=ALU.max, op1=ALU.min)
                    nc.vector.tensor_tensor(
                        out=sl(x_new), in0=sl(qv), in1=sl(cth),
                        op=ALU.subtract)
                    if ck is not None:
                        nc.vector.tensor_tensor(
                            out=sl(yth), in0=sl(x_new), in1=sl(x_old),
                            op=ALU.subtract)
                        nc.vector.scalar_tensor_tensor(
                            out=sl(yth), in0=sl(yth), scalar=ck,
                            in1=sl(x_new), op0=ALU.mult, op1=ALU.add)
                    if out_dma:
                        nc.sync.dma_start(
                            out_d[:, g * 4 * BC:(g + 1) * 4 * BC], sl(x_new))

            # ---------------- iteration 0 (y == 0) ----------------
            # res = yin ; z = src*yin ; vth = m2 ; vdl = yin/L
            for g in range(4):
                nc.vector.tensor_tensor(
                    out=z16[:, 4 * g:4 * g + 4, :],
                    in0=srcT[:, 4 * g:4 * g + 4, :],
                    in1=yinT[:, 4 * g:4 * g + 4, :], op=ALU.mult)
            pm2 = [pp.tile([128, 4 * BC], f32, name=f"p2{i}", tag=f"pm2_{i}")
                   for i in range(4)]
            for h in range(2):
                mm_half(pm2, wc_sb, z16, h)

            # delta half (DVE; GPSIMD elementwise is ~20x too slow and its
            # SBUF-port contention also stalls DVE)
            for h in range(2):
                nc.vector.tensor_scalar(
                    out=halfsl(vdl, h), in0=halfsl(yinT, h),
                    scalar1=invL, scalar2=None, op0=ALU.mult)
                nc.vector.tensor_scalar(
                    out=halfsl(cdl, h), in0=halfsl(vdl, h),
                    scalar1=-thresh, scalar2=thresh, op0=ALU.max, op1=ALU.min)
                nc.vector.tensor_tensor(
                    out=halfsl(xdlB, h), in0=halfsl(vdl, h),
                    in1=halfsl(cdl, h), op=ALU.subtract)
                # y1 = x1 (c0 = 0)
                nc.vector.tensor_copy(out=halfsl(ydl, h), in_=halfsl(xdlB, h))
                nc.vector.tensor_tensor(
                    out=halfsl(ynd, h), in0=halfsl(yinT, h),
                    in1=halfsl(ydl, h), op=ALU.subtract)

            theta_eval(pm2, xthB, first=True)
            for h in range(2):
                nc.vector.tensor_copy(out=halfsl(yth, h), in_=halfsl(xthB, h))

            x_old = [xthB, xdlB]
            x_new = [xthA, xdlA]

            # ---------------- iterations 1..15 ----------------
            for k in range(1, ITERS):
                ck = cks[k]
                last = (k == ITERS - 1)

                pm1 = [pp.tile([128, 4 * BC], f32, name=f"p1{i}", tag=f"pm1_{i}")
                       for i in range(4)]
                for h in range(2):
                    mm_half(pm1, wr_sb, yth, h)
                    # q = src*m1 ; res = ynd - q ; z = src*res
                    for i in (2 * h, 2 * h + 1):
                        nc.vector.tensor_tensor(
                            out=qv[:, 4 * i:4 * i + 4, :],
                            in0=srcT[:, 4 * i:4 * i + 4, :],
                            in1=pm1[i][:], op=ALU.mult)
                    nc.vector.tensor_tensor(
                        out=halfsl(res, h), in0=halfsl(ynd, h),
                        in1=halfsl(qv, h), op=ALU.subtract)
                    nc.vector.tensor_tensor(
                        out=halfsl(z16, h), in0=halfsl(srcT, h),
                        in1=halfsl(res, h), op=ALU.mult)

                pm2 = [pp.tile([128, 4 * BC], f32, name=f"p2{i}", tag=f"pm2_{i}")
                       for i in range(4)]
                for h in range(2):
                    mm_half(pm2, wc_sb, z16, h)

                # delta half (DVE, overlaps m2 on PE)
                for h in range(2):
                    nc.vector.scalar_tensor_tensor(
                        out=halfsl(vdl, h), in0=halfsl(res, h), scalar=invL,
                        in1=halfsl(ydl, h), op0=ALU.mult, op1=ALU.add)
                    nc.vector.tensor_scalar(
                        out=halfsl(cdl, h), in0=halfsl(vdl, h),
                        scalar1=-thresh, scalar2=thresh,
                        op0=ALU.max, op1=ALU.min)
                    nc.vector.tensor_tensor(
                        out=halfsl(x_new[1], h), in0=halfsl(vdl, h),
                        in1=halfsl(cdl, h), op=ALU.subtract)
                if not last:
                    for h in range(2):
                        nc.vector.tensor_tensor(
                            out=halfsl(ydl, h), in0=halfsl(x_new[1], h),
                            in1=halfsl(x_old[1], h), op=ALU.subtract)
                        nc.vector.scalar_tensor_tensor(
                            out=halfsl(ydl, h), in0=halfsl(ydl, h), scalar=ck,
                            in1=halfsl(x_new[1], h), op0=ALU.mult, op1=ALU.add)
                        nc.vector.tensor_tensor(
                            out=halfsl(ynd, h), in0=halfsl(yinT, h),
                            in1=halfsl(ydl, h), op=ALU.subtract)
                else:
                    # final xdl DMA overlaps the remaining theta work
                    nc.sync.dma_start(out_d[:, NCH * BC:], x_new[1][:])

                theta_eval(pm2, x_new[0], first=False,
                           x_old=x_old[0], ck=None if last else ck,
                           out_dma=last)

                x_old, x_new = x_new, x_old

    nc.finalize()
    return nc


_CACHE = {}


def _pack(a, nch, dtype=None):
    # [nch*128, X] -> [128, nch, X] with partition = row mod 128
    import ml_dtypes
    x = np.ascontiguousarray(a).reshape(nch, 128, a.shape[1])
    out = np.ascontiguousarray(
        x.transpose(1, 0, 2).reshape(128, nch * a.shape[1]))
    return out.astype(dtype if dtype is not None else ml_dtypes.bfloat16)


def kernel(src, Y, W, alpha):
    src = np.asarray(src)
    Y = np.asarray(Y)
    W = np.asarray(W)
    alpha = np.asarray(alpha)

    from concourse.bass_utils import run_bass_kernel_spmd

    G = W.astype(np.float64).T @ W.astype(np.float64)
    L = float(np.linalg.eigvalsh(G)[-1])
    invL = float(np.float32(1.0 / L))
    thresh = float(np.float32(float(alpha.reshape(-1)[0]) / L * 0.5))
    cks = _momentum_coeffs(ITERS)

    key = (invL, thresh)
    if key not in _CACHE:
        _CACHE[key] = _build(invL, thresh, cks)
    nc = _CACHE[key]

    wrP = _pack(np.ascontiguousarray(W.T).astype(np.float32), NCH)
    wcP = _pack((W / L).astype(np.float32), NCH)
    src2 = src.reshape(B, S).astype(np.float32)
    Y2 = Y.reshape(B, S).astype(np.float32)

    in_maps = []
    for c in range(NCORES):
        sl = slice(c * BC, (c + 1) * BC)
        in_maps.append({
            "src": _pack(np.ascontiguousarray(src2[sl].T), NCH, np.float32),
            "yin": _pack(np.ascontiguousarray(Y2[sl].T), NCH),
            "wr": wrP,
            "wc": wcP,
        })

    r = run_bass_kernel_spmd(nc, in_maps, core_ids=list(range(NCORES)))
    global LAST_RESULTS
    LAST_RESULTS = r

    outs = []
    for c in range(NCORES):
        o = np.asarray(r.results[c]["out"]).astype(np.float32)
        o = o.reshape(128, 2, NCH, BC).transpose(1, 2, 0, 3).reshape(2 * S, BC)
        outs.append(o.T)  # [BC, 2S]
    out = np.concatenate(outs, axis=0)
    return out.reshape(B, 2 * S, 1).astype(np.float32)


LAST_RESULTS = None
